# revision 60
# baseline (speedup 1.0000x reference)
"""Trainium2 Bass kernel for nn_ConsciousnessMonitor (histogram_binning).

kernel(**inputs) takes FULL unsharded numpy inputs, returns the full (9,)
float32 output. Shards state_history along time across 8 NeuronCores.

Stage A streams HT t-major ([NCH][D][128] host layout, 512B line-rate
granules, ~360 GB/s): each 1MB block's S values complete immediately via
16 fp32 matmuls (ht block stationary, mask matrix moving) accumulated
into one PSUM tile, so the running min/max hides under the stream (the
last block is split in two so the PE tail is tiny). PSUM note: start=True
resets the whole bank on real HW, so the tile is zeroed once and every
matmul accumulates with start=False. Min/max -> partition_all_reduce ->
AllReduce(max). The affine bin transform broadcasts ds/b1 across
partitions (gpsimd partition_broadcast) and runs on DVE with the int
conversion fused into the scalar_tensor_tensor write; dense bf16 one-hots
(80 cols/chunk, no memset) feed one packed [40,40] PSUM histogram
accumulation -> single [40,40] DMA -> AllReduce(add); bin lanes are
s-major and one-hot blocks b-major so every one-hot operand is 2-byte
packed (DVE 2x_1p mode). The MI tail works
in count space on the full 40x40 joint: full rowsum = 4R (1/4 and 1/T^2
folded into the host-side block-diagonal selB), cross-block cells are
finite and masked out of the weighted sum. The differentiation branch is
gated behind collective-1's output DMA so it fills the collective
windows; tanh is computed via exp (x >= 0 so it underflows, never
overflows) and a dummy Ln preloads the table off the critical path.

Self-contained: shapes/sharding hardcoded; reads no sibling files.
"""
import numpy as np

import concourse.bacc as bacc
import concourse.tile as tile
import concourse.mybir as mybir
from concourse.bass_utils import run_bass_kernel_spmd
import concourse.bass_isa as bass_isa

F32 = mybir.dt.float32
F32R = mybir.dt.float32r
BF16 = mybir.dt.bfloat16
I32 = mybir.dt.int32
I16 = mybir.dt.int16
AX = mybir.AxisListType
OP = mybir.AluOpType
ACT = mybir.ActivationFunctionType

N_CORES = 8
T, D = 32768, 2048
TL = T // N_CORES          # 4096 time steps per core
NB = 10                    # histogram bins per axis
NPAIR = 4                  # partitions (mask pairs)
J = 2 * NPAIR              # 8 masked-mean columns
NDC = D // 128             # 16 contraction chunks
NCH = TL // 128            # 32 t-chunks of 128
CW = 2 * NPAIR * NB        # 80 one-hot cols per chunk (x pack + y pack)
MEM = 100
SN = 10
TINV = float(1.0 / (np.float32(T) + np.float32(1e-10)))

_CACHE = {}
LAST_RESULTS = None


def _build(debug=False, variant="main"):
    return _build_inner(debug, variant)


def _build_inner(debug, variant):
    sim1 = variant.startswith("sim1")
    nc = bacc.Bacc("TRN2", target_bir_lowering=False, debug=False,
                   num_devices=1 if sim1 else N_CORES)
    ht = nc.dram_tensor("ht", [NCH * D, 128], F32,
                        kind="ExternalInput").ap()
    msb = nc.dram_tensor("msb", [128, NDC * J], F32,
                         kind="ExternalInput").ap()
    invc2 = nc.dram_tensor("invc2", [1, 2 * J], F32,
                           kind="ExternalInput").ap()
    memsb = nc.dram_tensor("memsb", [128, NDC * MEM], F32,
                           kind="ExternalInput").ap()
    sampsb = nc.dram_tensor("sampsb", [128, NDC * SN], F32,
                            kind="ExternalInput").ap()
    selB = nc.dram_tensor("selB", [NPAIR * NB, NPAIR * NB], F32,
                          kind="ExternalInput").ap()
    maskc = nc.dram_tensor("maskc", [NPAIR * NB, NPAIR * NB], F32,
                           kind="ExternalInput").ap()
    selcR = nc.dram_tensor("selcR", [NPAIR * NB, NPAIR], F32,
                           kind="ExternalInput").ap()
    out = nc.dram_tensor("out", [9], F32, kind="ExternalOutput").ap()
    dbg = variant == "dbg"
    if dbg:
        dbg_gmm = nc.dram_tensor("dbg_gmm", [1, 2 * J], F32,
                                 kind="ExternalOutput").ap()
        dbg_row = nc.dram_tensor("dbg_row", [1, 2 * J], F32,
                                 kind="ExternalOutput").ap()
        dbg_bin = nc.dram_tensor("dbg_bin", [128, NCH * J], I16,
                                 kind="ExternalOutput").ap()
        dbg_gj = nc.dram_tensor("dbg_gj", [NPAIR * NB, NPAIR * NB], F32,
                                kind="ExternalOutput").ap()

    rg = [list(range(N_CORES))]

    with tile.TileContext(nc) as tc:
        with tc.tile_pool(name="consts", bufs=1) as consts, \
             tc.tile_pool(name="sb", bufs=1) as sb, \
             tc.tile_pool(name="htp", bufs=3) as htp, \
             tc.tile_pool(name="pst", bufs=1, space="PSUM") as pst, \
             tc.tile_pool(name="misc", bufs=3, space="PSUM") as misc, \
             tc.tile_pool(name="ps2", bufs=2, space="PSUM") as ps2, \
             tc.tile_pool(name="dram", bufs=1, space="DRAM") as dram:

            # ---- constants on the gpsimd queue so the ht stream owns the
            # sync queue from t=0 ----
            m_sb = consts.tile([128, NDC * J], F32, tag="msb")
            nc.gpsimd.dma_start(out=m_sb[:], in_=msb[:])
            invc2_sb = consts.tile([1, 2 * J], F32, tag="invc2")
            nc.gpsimd.dma_start(out=invc2_sb[:], in_=invc2[:])
            selB_sb = consts.tile([NPAIR * NB, NPAIR * NB], F32, tag="selb")
            nc.gpsimd.dma_start(out=selB_sb[:], in_=selB[:])
            maskc_sb = consts.tile([NPAIR * NB, NPAIR * NB], F32,
                                   tag="maskc")
            nc.gpsimd.dma_start(out=maskc_sb[:], in_=maskc[:])
            selcR_sb = consts.tile([NPAIR * NB, NPAIR], F32, tag="selcr")
            nc.gpsimd.dma_start(out=selcR_sb[:], in_=selcR[:])

            ones128 = consts.tile([128, 1], F32, tag="o128")
            nc.gpsimd.memset(ones128[:], 1.0)
            ones1_10 = consts.tile([1, NB], F32, tag="o110")
            nc.gpsimd.memset(ones1_10[:], 1.0)
            ones10 = consts.tile([NB, 1], F32, tag="o10")
            nc.gpsimd.memset(ones10[:], 1.0)
            io16 = consts.tile([128, NPAIR * NB], I16, tag="io16")
            nc.gpsimd.iota(io16[:], pattern=[[1, NB], [0, NPAIR]], base=0,
                           channel_multiplier=0)

            # ---- stage A: stream HT t-major ([NCH][D][128] host layout,
            # 512B granules = line-rate); each 1MB block completes its S
            # values immediately so the running min/max hides under the
            # stream. A matmul with start=True resets the WHOLE PSUM bank
            # on real HW, so 32 interleaved accumulation groups in one bank
            # would corrupt each other: zero the bank once and accumulate
            # with start=False everywhere instead ----
            psS = pst.tile([128, NCH * J], F32, tag="pss", name="psS")
            nc.vector.memset(psS[:], 0.0)
            mmB = sb.tile([128, 2 * J], F32, tag="mmb")
            nc.gpsimd.memset(mmB[:], -1e30)
            # dummy op hoists the gpsimd lib-1 ucode reload off the
            # critical path (partition_all_reduce/broadcast library)
            papre = consts.tile([128, 1], F32, tag="papre")
            nc.gpsimd.partition_all_reduce(papre[:], ones128[:], 128,
                                           bass_isa.ReduceOp.max)
            for tcn in range(NCH):
                htb = htp.tile([128, NDC * 128], F32, tag="htb", name="htb")
                if tcn < NCH - 1:
                    nc.sync.dma_start(
                        out=htb[:].rearrange("p (k t) -> p k t", t=128),
                        in_=ht[tcn * D:(tcn + 1) * D, :].rearrange(
                            "(k p) t -> p k t", p=128))
                else:
                    # split the last block so PE starts on its first half
                    # ~1.5us before the stream's last byte
                    HD = D // 2
                    for h in range(2):
                        nc.sync.dma_start(
                            out=htb[:, h * HD:(h + 1) * HD].rearrange(
                                "p (k t) -> p k t", t=128),
                            in_=ht[tcn * D + h * HD:tcn * D + (h + 1) * HD,
                                   :].rearrange("(k p) t -> p k t", p=128))
                for k in range(NDC):
                    nc.tensor.matmul(
                        psS[:, tcn * J:(tcn + 1) * J],
                        htb[:, k * 128:(k + 1) * 128],
                        m_sb[:, k * J:(k + 1) * J],
                        start=False, stop=(k == NDC - 1),
                        skip_group_check=True)
                blk = psS[:, tcn * J:(tcn + 1) * J]
                nc.vector.tensor_tensor(mmB[:, 0:J], mmB[:, 0:J], blk,
                                        OP.max)
                nc.vector.scalar_tensor_tensor(mmB[:, J:2 * J], blk, -1.0,
                                               mmB[:, J:2 * J], OP.mult,
                                               OP.max)

            # ---- cross-core AllReduce of the min/max ----
            mmA = sb.tile([128, 2 * J], F32, tag="mma")
            nc.gpsimd.partition_all_reduce(mmA[:], mmB[:], 128,
                                           bass_isa.ReduceOp.max)
            cbA = dram.tile([1, 2 * J], F32, tag="cba")
            cbB = dram.tile([1, 2 * J], F32, tag="cbb")
            nc.sync.dma_start(out=cbA[:], in_=mmA[0:1, :])
            if sim1:
                nc.gpsimd.dma_start(out=cbB[:], in_=cbA[:])
            else:
                nc.gpsimd.collective_compute("AllReduce", OP.max,
                                             replica_groups=rg,
                                             ins=[cbA.opt()],
                                             outs=[cbB.opt()])
            # diff-branch inputs queued on sync AFTER cbA: they land (and
            # the branch runs) inside the collective window
            mem_sb = consts.tile([128, NDC * MEM], F32, tag="memsb")
            nc.sync.dma_start(out=mem_sb[:], in_=memsb[:])
            samp_sb = consts.tile([128, NDC * SN], F32, tag="sampsb")
            nc.sync.dma_start(out=samp_sb[:], in_=sampsb[:])
            gmm = sb.tile([1, 2 * J], F32, tag="gmm")
            nc.sync.dma_start(out=gmm[:], in_=cbB[:])

            # ---- differentiation branch part 1 (hidden in collective-1
            # window: DVE is idle 98.6-104.9, Act idle until the jm copies;
            # msum goes through Activation accumulate so the DVE work fits
            # the window) ----
            psG = misc.tile([SN, SN], F32, tag="m", name="psG")
            for k in range(NDC):
                nc.tensor.matmul(psG[:], samp_sb[:, k * SN:(k + 1) * SN],
                                 samp_sb[:, k * SN:(k + 1) * SN],
                                 start=(k == 0), stop=(k == NDC - 1))
            sqs = sb.tile([128, NDC * SN], F32, tag="sqs")
            nc.vector.tensor_tensor(sqs[:], samp_sb[:], samp_sb[:], OP.mult)
            psrc = misc.tile([SN, 1], F32, tag="m", name="psrc")
            psrr = misc.tile([1, SN], F32, tag="m", name="psrr")
            for k in range(NDC):
                nc.tensor.matmul(psrc[:], sqs[:, k * SN:(k + 1) * SN],
                                 ones128[:], start=(k == 0),
                                 stop=(k == NDC - 1))
            for k in range(NDC):
                nc.tensor.matmul(psrr[:], ones128[:],
                                 sqs[:, k * SN:(k + 1) * SN],
                                 start=(k == 0), stop=(k == NDC - 1))
            g_sb = sb.tile([SN, SN], F32, tag="gsb")
            nc.scalar.copy(g_sb[:], psG[:])
            rcol = sb.tile([SN, 1], F32, tag="rcol")
            nc.scalar.copy(rcol[:], psrc[:])
            rrow = sb.tile([1, SN], F32, tag="rrow")
            nc.scalar.copy(rrow[:], psrr[:])
            rB = misc.tile([SN, SN], F32, tag="m", name="rB")
            nc.tensor.matmul(rB[:], ones1_10[:], rrow[:], start=True,
                             stop=True)
            d2 = sb.tile([SN, SN], F32, tag="d2")
            nc.vector.scalar_tensor_tensor(d2[:], g_sb[:], -2.0, rB[:],
                                           OP.mult, OP.add)
            nc.vector.tensor_scalar(d2[:], d2[:], rcol[:], 1e-20, OP.add,
                                    OP.max)
            dst = sb.tile([SN, SN], F32, tag="dst")
            nc.scalar.activation(dst[:], d2[:], ACT.Sqrt)
            dsum = sb.tile([SN, 1], F32, tag="dsum")
            nc.vector.tensor_reduce(dsum[:], dst[:], AX.X, OP.add)
            psD = misc.tile([1, 1], F32, tag="m", name="psD")
            nc.tensor.matmul(psD[:], dsum[:], ones10[:], start=True,
                             stop=True)
            avg_sb = sb.tile([1, 1], F32, tag="avgsb")
            nc.vector.tensor_scalar(avg_sb[:], psD[:],
                                    float(1.0 / (SN * (SN - 1) + 1e-6)),
                                    None, OP.mult)

            msum = sb.tile([128, NDC], F32, tag="msum")
            s2sum = sb.tile([128, NDC], F32, tag="s2sum")
            sqm = sb.tile([128, NDC * MEM], F32, tag="sqm")
            nc.vector.tensor_reduce(
                msum[:], mem_sb[:].rearrange("p (k f) -> p k f", f=MEM),
                AX.X, OP.add)
            nc.vector.tensor_tensor(sqm[:], mem_sb[:], mem_sb[:], OP.mult)
            nc.vector.tensor_reduce(
                s2sum[:], sqm[:].rearrange("p (k f) -> p k f", f=MEM),
                AX.X, OP.add)
            # variance combine + tv/sqtv/exp/ln-preload all complete inside
            # the collective-1 window so the Act table loads never touch the
            # histogram path
            var16 = sb.tile([128, NDC], F32, tag="var16")
            nc.vector.tensor_tensor(var16[:], msum[:], msum[:], OP.mult)
            nc.vector.tensor_scalar(var16[:], var16[:],
                                    float(-1.0 / MEM), None, OP.mult)
            nc.vector.tensor_tensor(var16[:], var16[:], s2sum[:], OP.add)
            nc.vector.tensor_scalar(var16[:], var16[:],
                                    float(1.0 / (MEM - 1)), None, OP.mult)
            redv = sb.tile([128, 1], F32, tag="redv")
            nc.vector.tensor_reduce(redv[:], var16[:], AX.X, OP.add)
            v2 = sb.tile([128, NDC], F32, tag="v2")
            nc.vector.tensor_tensor(v2[:], var16[:], var16[:], OP.mult)
            redv2 = sb.tile([128, 1], F32, tag="redv2")
            nc.vector.tensor_reduce(redv2[:], v2[:], AX.X, OP.add)
            pstv = misc.tile([1, 1], F32, tag="m", name="pstv")
            nc.tensor.matmul(pstv[:], redv[:], ones128[:], start=True,
                             stop=True)
            pss2 = misc.tile([1, 1], F32, tag="m", name="pss2")
            nc.tensor.matmul(pss2[:], redv2[:], ones128[:], start=True,
                             stop=True)
            outrow = sb.tile([1, 9], F32, tag="outrow")
            tv_sb = sb.tile([1, 1], F32, tag="tvsb")
            nc.vector.tensor_copy(tv_sb[:], pstv[:])
            tvsq = sb.tile([1, 1], F32, tag="tvsq")
            nc.vector.tensor_tensor(tvsq[:], tv_sb[:], tv_sb[:], OP.mult)
            dden = sb.tile([1, 1], F32, tag="dden")
            nc.vector.scalar_tensor_tensor(dden[:], tvsq[:], 1e-6, pss2[:],
                                           OP.mult, OP.add)
            rdden = sb.tile([1, 1], F32, tag="rdden")
            nc.vector.reciprocal(rdden[:], dden[:])
            nc.vector.tensor_tensor(outrow[:, 2:3], tvsq[:], rdden[:],
                                    OP.mult)
            nc.vector.tensor_copy(outrow[:, 3:4], tv_sb[:])
            sqtv = sb.tile([1, 1], F32, tag="sqtv")
            nc.scalar.activation(sqtv[:], pstv[:], ACT.Sqrt)
            nc.vector.tensor_tensor(outrow[:, 1:2], sqtv[:], avg_sb[:],
                                    OP.mult)
            # tanh(x) = (1 - e^{-2x}) / (1 + e^{-2x}); x >= 0 here so the
            # exp underflows (never overflows)
            etan = sb.tile([1, 1], F32, tag="etan")
            nc.scalar.activation(etan[:], outrow[:, 1:2], ACT.Exp,
                                 scale=-2.0)
            tb = sb.tile([1, 1], F32, tag="tb")
            nc.vector.tensor_scalar(tb[:], etan[:], 1.0, None, OP.add)
            # prime the Ln table now (tb >= 1, value unused) so the MI Ln
            # needs no load later; consuming tb forces sqrt->exp->ln order
            # on the Act queue (no redundant set reloads)
            lnpre = sb.tile([1, 1], F32, tag="lnpre")
            nc.scalar.activation(lnpre[:], tb[:], ACT.Ln)

            # ---- post-collective-1: bin scale factors ----
            gms = sb.tile([1, 2 * J], F32, tag="gms")
            nc.vector.tensor_tensor(gms[:], gmm[:], invc2_sb[:], OP.mult)
            den = sb.tile([1, J], F32, tag="den")
            nc.vector.scalar_tensor_tensor(den[:], gms[:, 0:J], 1e-6,
                                           gms[:, J:2 * J], OP.add, OP.add)
            rr = sb.tile([1, J], F32, tag="rr")
            nc.vector.reciprocal(rr[:], den[:])
            dsb1row = sb.tile([1, 2 * J], F32, tag="dsb1row")
            nc.vector.scalar_tensor_tensor(dsb1row[:, 0:J], rr[:], 10.0,
                                           invc2_sb[:, 0:J], OP.mult,
                                           OP.mult)
            nc.vector.scalar_tensor_tensor(dsb1row[:, J:2 * J], rr[:], 10.0,
                                           gms[:, J:2 * J], OP.mult,
                                           OP.mult)
            dsb1 = sb.tile([128, 2 * J], F32, tag="dsb1")
            nc.gpsimd.partition_broadcast(dsb1[:], dsb1row[:])

            # ---- bin values + one-hot + packed joint histogram ----
            ps3 = psS[:].rearrange("p (c j) -> p c j", j=J)
            dsv = dsb1[:, 0:J].rearrange("p (c j) -> p c j", c=1)
            b1v = dsb1[:, J:2 * J].rearrange("p (c j) -> p c j", c=1)
            bin1 = sb.tile([128, NCH * J], F32, tag="bin1")
            bin13 = bin1[:].rearrange("p (c j) -> p c j", j=J)
            nc.vector.tensor_tensor(bin13, ps3,
                                    dsv.broadcast_to([128, NCH, J]), OP.mult)
            # (bin1 - 0.5 + b1) with round-to-nearest on the i32 write
            # (same rounding as tensor_copy), then integer clip
            binint = sb.tile([128, NCH * J], I16, tag="binint")
            bini3 = binint[:].rearrange("p (c j) -> p c j", j=J)
            nc.vector.scalar_tensor_tensor(bini3, bin13, -0.5,
                                           b1v.broadcast_to([128, NCH, J]),
                                           OP.add, OP.add)
            nc.vector.tensor_scalar(binint[:], binint[:], 0, NB - 1, OP.max,
                                    OP.min)
            # dense one-hot, bf16: per chunk 80 cols: x pack [p*10+b] then
            # y pack [40 + p*10+b]; fully written so no memset needed
            ohsb = sb.tile([128, NCH * CW], BF16, tag="ohsb")
            oh3 = ohsb[:].rearrange("pt (c r) -> pt c r", r=CW)
            ohx = oh3[:, :, 0:NPAIR * NB].rearrange(
                "pt c (b q) -> pt c b q", q=NPAIR)
            ohy = oh3[:, :, NPAIR * NB:2 * NPAIR * NB].rearrange(
                "pt c (b q) -> pt c b q", q=NPAIR)
            bi4 = binint[:].rearrange("pt (c s p) -> pt c s p", c=NCH,
                                      s=2, p=NPAIR)
            io4 = io16[:].rearrange("pt (x b q) -> pt x b q", x=1, q=NPAIR)
            psJ = ps2.tile([NPAIR * NB, NPAIR * NB], F32, tag="psj",
                           name="psJ")
            HALF = NCH // 2
            for h in range(2):
                c0, c1 = h * HALF, (h + 1) * HALF
                for s, dstv in ((0, ohx), (1, ohy)):
                    bi = bi4[:, c0:c1, s, :][:, :, None, :]
                    nc.vector.tensor_tensor(
                        dstv[:, c0:c1],
                        bi.broadcast_to([128, HALF, NB, NPAIR]),
                        io4.broadcast_to([128, HALF, NB, NPAIR]),
                        OP.is_equal)
                for c in range(c0, c1):
                    nc.tensor.matmul(
                        psJ[:],
                        ohsb[:, c * CW:c * CW + NPAIR * NB],
                        ohsb[:, c * CW + NPAIR * NB:(c + 1) * CW],
                        start=(c == 0), stop=(c == NCH - 1))
            # ship the whole [40,40] joint; cross-block cells are handled
            # exactly in the MI tail (full rowsum = 4R, blockdiag colsum = C)
            jm = sb.tile([NPAIR * NB, NPAIR * NB], F32, tag="jm")
            nc.vector.tensor_copy(jm[:], psJ[:])
            cbj = dram.tile([NPAIR * NB, NPAIR * NB], F32, tag="cbj")
            cbj2 = dram.tile([NPAIR * NB, NPAIR * NB], F32, tag="cbj2")
            nc.sync.dma_start(out=cbj[:], in_=jm[:])
            if sim1:
                nc.gpsimd.dma_start(out=cbj2[:], in_=cbj[:])
            else:
                nc.gpsimd.collective_compute("AllReduce", OP.add,
                                             replica_groups=rg,
                                             ins=[cbj.opt()],
                                             outs=[cbj2.opt()])

            # ---- tanh combine (tiny DVE ops, hidden in collective-2) ----
            ta = sb.tile([1, 1], F32, tag="ta")
            nc.vector.tensor_scalar(ta[:], etan[:], -1.0, 1.0, OP.mult,
                                    OP.add)
            trb = sb.tile([1, 1], F32, tag="trb")
            nc.vector.reciprocal(trb[:], tb[:])
            tanhd = sb.tile([1, 1], F32, tag="tanhd")
            nc.vector.tensor_tensor(tanhd[:], ta[:], trb[:], OP.mult)

            gj = sb.tile([NPAIR * NB, NPAIR * NB], F32, tag="gj")
            nc.sync.dma_start(out=gj[:], in_=cbj2[:])

            # ---- MI tail in count space on the full [40,40] joint.
            # Full rowsum = 4R (each y-block contributes R once; /4 folded
            # into selB); selB blockdiag colsum gives C*TINV^2/4 for every
            # column; cross-block cells get finite garbage lg that is
            # zeroed by the blockdiag mask before the weighted sum. ----
            rowsum = sb.tile([NPAIR * NB, 1], F32, tag="rowsum")
            nc.vector.tensor_reduce(rowsum[:], gj[:], AX.X, OP.add)
            num = sb.tile([NPAIR * NB, NPAIR * NB], F32, tag="num")
            nc.vector.tensor_scalar(num[:], gj[:], TINV, 1e-10, OP.mult,
                                    OP.add)
            psOut = misc.tile([NPAIR * NB, NPAIR * NB], F32, tag="m",
                              name="psOut")
            nc.tensor.matmul(psOut[:], selB_sb[:], gj[:], start=True,
                             stop=True)
            outer = sb.tile([NPAIR * NB, NPAIR * NB], F32, tag="outer")
            nc.vector.tensor_scalar(outer[:], psOut[:], rowsum[:], 1e-10,
                                    OP.mult, OP.add)
            rout = sb.tile([NPAIR * NB, NPAIR * NB], F32, tag="rout")
            nc.vector.reciprocal(rout[:], outer[:])
            nc.vector.tensor_tensor(num[:], num[:], rout[:], OP.mult)
            lg = sb.tile([NPAIR * NB, NPAIR * NB], F32, tag="lg")
            nc.scalar.activation(lg[:], num[:], ACT.Ln)
            # gjm fills the DVE idle window while Ln runs on Act
            gjm = sb.tile([NPAIR * NB, NPAIR * NB], F32, tag="gjm")
            nc.vector.tensor_tensor(gjm[:], gj[:], maskc_sb[:], OP.mult)
            nc.vector.tensor_tensor(lg[:], gjm[:], lg[:], OP.mult)
            ms = sb.tile([NPAIR * NB, 1], F32, tag="ms")
            nc.vector.tensor_reduce(ms[:], lg[:], AX.X, OP.add)
            psRow = misc.tile([1, NPAIR], F32, tag="m", name="psRow")
            nc.tensor.matmul(psRow[:], ms[:], selcR_sb[:], start=True,
                             stop=True)
            # min commutes with max(.,0): integration = max(min(mi), 0)
            mimin = sb.tile([1, 1], F32, tag="mimin")
            nc.vector.tensor_reduce(mimin[:], psRow[:], AX.X, OP.min)
            nc.vector.tensor_scalar(outrow[:, 4:5], mimin[:], 0.0, None,
                                    OP.max)
            nc.vector.scalar_tensor_tensor(outrow[:, 0:1], mimin[:], 0.0,
                                           tanhd[:], OP.max, OP.add)
            nc.vector.tensor_scalar(outrow[:, 5:9], psRow[:], 0.0, None,
                                    OP.max)
            nc.sync.dma_start(out=out[:], in_=outrow[:])
            if dbg:
                nc.sync.dma_start(out=dbg_gmm[:], in_=gmm[:])
                nc.sync.dma_start(out=dbg_row[:], in_=dsb1row[:])
                nc.sync.dma_start(out=dbg_bin[:], in_=binint[:])
                nc.sync.dma_start(out=dbg_gj[:], in_=gj[:])

    nc.compile()
    return nc


def _get_nc(debug=False):
    key = ("ncd" if debug else "nc")
    if key not in _CACHE:
        _CACHE[key] = _build(debug)
    return _CACHE[key]


def kernel(state, state_memory, state_history, partitions, sample_idx,
           trace=False, debug=False):
    global LAST_RESULTS
    state = np.asarray(state, np.float32)
    state_memory = np.asarray(state_memory, np.float32)
    state_history = np.asarray(state_history, np.float32)
    partitions = np.asarray(partitions)
    sample_idx = np.asarray(sample_idx)

    # mask columns s-major (j = s*4 + p) so the bin-lane pair index is
    # stride-1 for the packed 2x one-hot
    mmat = np.empty((D, J), np.float32)
    invc8 = np.empty((J,), np.float32)
    pf = partitions.astype(np.float32)
    for p in range(NPAIR):
        mmat[:, p] = pf[p]
        mmat[:, NPAIR + p] = np.float32(1.0) - pf[p]
        invc8[p] = np.float32(1.0) / pf[p].sum(dtype=np.float32)
        invc8[NPAIR + p] = np.float32(1.0) / (np.float32(1.0)
                                              - pf[p]).sum(dtype=np.float32)
    invc2 = np.concatenate([invc8, invc8]).reshape(1, 2 * J)
    # SBUF layouts precomputed host-side: [128, k*cols] with row d = k*128+p
    msb = np.ascontiguousarray(
        mmat.reshape(NDC, 128, J).transpose(1, 0, 2).reshape(128, NDC * J))
    memory = np.concatenate([state, state_memory[state.shape[0]:]], axis=0)
    memsb = np.ascontiguousarray(
        memory.T.reshape(NDC, 128, MEM).transpose(1, 0, 2).reshape(
            128, NDC * MEM))
    sampsb = np.ascontiguousarray(
        memory[sample_idx].T.reshape(NDC, 128, SN).transpose(1, 0, 2).reshape(
            128, NDC * SN))
    # histogram cols are b-major (col = b*4 + p): pair(col) = col % 4
    pair_of = np.arange(NPAIR * NB) % NPAIR
    selcm = (pair_of[:, None] == np.arange(NPAIR)[None, :]).astype(
        np.float32)
    selcR = np.ascontiguousarray(selcm * np.float32(TINV))
    maskc = (pair_of[:, None] == pair_of[None, :]).astype(np.float32)
    selB = np.ascontiguousarray(maskc * np.float32(TINV * TINV / 4.0))
    maskc = np.ascontiguousarray(maskc)

    in_maps = []
    for c in range(N_CORES):
        htc = np.ascontiguousarray(
            state_history[c * TL:(c + 1) * TL, :].reshape(
                NCH, 128, D).transpose(0, 2, 1).reshape(NCH * D, 128))
        in_maps.append({"ht": htc, "msb": msb, "invc2": invc2,
                        "memsb": memsb, "sampsb": sampsb,
                        "selB": selB, "maskc": maskc, "selcR": selcR})

    nc = _get_nc(debug)
    res = run_bass_kernel_spmd(nc, in_maps, list(range(N_CORES)),
                               trace=trace)
    LAST_RESULTS = res
    return np.asarray(res.results[0]["out"], np.float32)
